# revision 42
# baseline (speedup 1.0000x reference)
"""Trainium2 Bass kernel for nn_BlockAttentionResidual.

Transformer block: RMSNorm -> QKV -> RoPE -> block-diagonal causal attention
(4 blocks of 512) -> o-proj + residual -> RMSNorm -> SwiGLU FFN + residual.
Shapes: x [2, 2048, 2048], 32 heads x 64, inter 4096.

Sharding: 8 cores = (batch 2) x (4 sequence blocks of 512 tokens). The
attention mask is block-diagonal causal with block size 512, so each core's
512-token slice is fully independent -> no collectives.

On-device layout is "T layout" [feature, token] throughout, because every
matmul contracts the feature dim, which must sit on SBUF partitions for the
PE. Matmuls run in bf16 with fp32 PSUM accumulation; softmax skips the max
subtraction (scores are small); the causal mask is only needed on the
128x128 diagonal chunks of each 512 block; softmax denominators come from a
ones-column appended to V; per-token broadcasts across partitions are done
as rank-1 outer-product matmuls on the PE.
"""

import math
from contextlib import ExitStack

import ml_dtypes
import numpy as np

import concourse.bass as bass
import concourse.mybir as mybir
import concourse.tile as tile
from concourse.bass_utils import run_bass_kernel_spmd
from concourse.vector_clock import ScopedClock

F32 = mybir.dt.float32
BF16 = mybir.dt.bfloat16
NPBF16 = ml_dtypes.bfloat16

EPS = 1e-5
ROPE_THETA = 10000.0


# --- workaround: this walrus build allows only one sem wait per CTRL-queue
# instruction (Drain/NoOp), but Tile's tail drain aggregates every
# outstanding wait onto a single SP Drain. Spread them over SP NOPs.
def _patched_drain_and_barrier(self, tick_clock, wait_clock):
    nop_inst = self.nc.sync.nop(nofuse=True)
    wait_clock.add_sem_waits(
        nop_inst.ins, ScopedClock({None: tick_clock.global_clock})
    )
    si = nop_inst.ins.sync_info
    waits = list(si.on_wait) if si is not None else []
    if len(waits) > 1:
        si.on_wait = waits[:1]
        for w in waits[1:]:
            n2 = self.nc.sync.nop(nofuse=True)
            if n2.ins.sync_info is None:
                n2.ins.sync_info = mybir.SyncInfo(on_wait=[w], on_update=[])
            else:
                n2.ins.sync_info.on_wait = [w]
    self.nc.sync.drain()
    self.nc.all_engine_barrier()
    assert self.sems is not None
    popped = self.nc._tile_sem_poison_stack.pop()
    assert popped is self._sem_poison
    self.nc.clear_and_free_semaphores(list(self.sems.allocated().values()))
    self.nc.all_engine_barrier()


tile.TileContext._drain_and_barrier = _patched_drain_and_barrier


def _split_excess_waits(nc, maxw=1):
    """This walrus build rejects instructions carrying more than one sync
    wait; hoist extras onto single-wait NOPs queued just before on the same
    engine."""
    fn = nc.m.functions[0]
    for bb in fn.blocks:
        out = []
        changed = False
        for inst in bb.instructions:
            si = getattr(inst, "sync_info", None)
            waits = list(si.on_wait) if si is not None else []
            if len(waits) > maxw:
                changed = True
                for w in waits[:-maxw]:
                    nop = mybir.InstNoOp(
                        name=nc.get_next_instruction_name(), ins=[], outs=[])
                    nop.engine = inst.engine
                    nop.sync_info = mybir.SyncInfo(on_wait=[w], on_update=[])
                    out.append(nop)
                si.on_wait = waits[-maxw:]
            out.append(inst)
        if changed:
            bb.instructions = out


class Cfg:
    def __init__(self, T=512, C=2048, H=32, D=64, I=4096):
        self.T = T          # tokens per core (one attention block)
        self.C = C          # hidden
        self.H = H          # heads
        self.D = D          # head dim (must be 64)
        self.I = I          # ffn inner
        assert D == 64 and C == H * D
        assert T % 128 == 0 and C % 128 == 0 and I % 128 == 0


def build_program(cfg: Cfg):
    T, C, H, D, I = cfg.T, cfg.C, cfg.H, cfg.D, cfg.I
    CT = C // 128            # hidden tiles
    KT = T // 128            # token chunks (and attention k-chunks)
    QKN = 2 * C              # q+k feature rows
    ICH = I // 128           # ffn inner chunks
    scale = 1.0 / math.sqrt(D)
    OG = 8                   # psum group width for dense matmul phases

    nc = bass.Bass("TRN2", target_bir_lowering=False, debug=False)

    xT = nc.dram_tensor("xT", (C, T), F32, kind="ExternalInput").ap()
    w_qkT = nc.dram_tensor("w_qkT", (C, QKN), BF16, kind="ExternalInput").ap()
    w_vT = nc.dram_tensor("w_vT", (C, C), BF16, kind="ExternalInput").ap()
    w_oT = nc.dram_tensor("w_oT", (C, C), BF16, kind="ExternalInput").ap()
    w_upT = nc.dram_tensor("w_upT", (C, 2 * I), BF16, kind="ExternalInput").ap()
    w_downT = nc.dram_tensor("w_downT", (I, C), BF16, kind="ExternalInput").ap()
    cosT2 = nc.dram_tensor("cosT2", (128, T), BF16, kind="ExternalInput").ap()
    nsinT2 = nc.dram_tensor("nsinT2", (128, T), BF16, kind="ExternalInput").ap()
    trimask = nc.dram_tensor("trimask", (128, 128), BF16, kind="ExternalInput").ap()
    pswap = nc.dram_tensor("pswap", (128, 128), BF16, kind="ExternalInput").ap()
    outT = nc.dram_tensor("outT", (C, T), F32, kind="ExternalOutput").ap()

    with tile.TileContext(nc) as tc, ExitStack() as ctx:
        consts = ctx.enter_context(tc.tile_pool(name="consts", bufs=1))
        # xt slots also serve x2 (x dies at the o-proj residual add); +2
        # rolling slots so the x2 alloc never waits on its own free.
        xt_pool = ctx.enter_context(tc.tile_pool(name="xt", bufs=CT + 2))
        ht_pool = ctx.enter_context(tc.tile_pool(name="ht", bufs=CT))
        qk_pool = ctx.enter_context(
            tc.tile_pool(name="qk", bufs=max(2 * CT, ICH)))
        v_pool = ctx.enter_context(tc.tile_pool(name="v", bufs=KT))
        ctx_pool = ctx.enter_context(tc.tile_pool(name="ctx", bufs=CT))
        wsl_pool = ctx.enter_context(tc.tile_pool(name="wsl", bufs=8))
        tr_pool = ctx.enter_context(tc.tile_pool(name="tr", bufs=4))
        e_pool = ctx.enter_context(tc.tile_pool(name="e", bufs=8))
        sm_pool = ctx.enter_context(tc.tile_pool(name="sm", bufs=2))
        rsrc_pool = ctx.enter_context(tc.tile_pool(name="rsrc", bufs=9))
        ps_pool = ctx.enter_context(
            tc.tile_pool(name="ps", bufs=8, space="PSUM"))

        _nm = [0]

        def named(base):
            _nm[0] += 1
            return f"{base}{_nm[0]}"

        def ps_tile():
            return ps_pool.tile([128, T], F32, tag="ps", name=named("ps"))

        # ---- load xT first: the first sumsq matmul waits on xt[0], so
        # its DMA must lead the issue queues (constants aren't needed until
        # RoPE, ~40us in)
        xt = []
        for ci in range(CT):
            t = xt_pool.tile([128, T], F32, tag="xt", name=named("t"))
            nc.sync.dma_start(t[:], xT[ci * 128:(ci + 1) * 128, :])
            xt.append(t)

        # ---- constants
        sb_cos = consts.tile([128, T], BF16)
        nc.sync.dma_start(sb_cos[:], cosT2[:])
        sb_nsin = consts.tile([128, T], BF16)
        nc.sync.dma_start(sb_nsin[:], nsinT2[:])
        sb_tri = consts.tile([128, 128], BF16)
        nc.sync.dma_start(sb_tri[:], trimask[:])
        sb_psw = consts.tile([128, 128], BF16)
        nc.sync.dma_start(sb_psw[:], pswap[:])
        ones_col = consts.tile([128, 1], BF16)
        nc.vector.memset(ones_col[:], 1.0)
        ones_row = consts.tile([1, 128], BF16)
        nc.vector.memset(ones_row[:], 1.0)
        ones_rows = consts.tile([128, 64], BF16)
        nc.vector.memset(ones_rows[:], 1.0)
        eps_t = consts.tile([1, 1], F32)
        nc.vector.memset(eps_t[:], EPS)


        def rmsnorm(x_tiles, nw_row, out_tag, ps_ss=None):
            """x_tiles: CT fp32 [128, T] tiles (T layout) -> bf16 tiles of
            x * rstd[t] (the norm weight is folded into the next projection
            on the host). ps_ss: optionally a psum tile already holding the
            sum of squares."""
            if ps_ss is None:
                ps_ss = ps_tile()
                for ci in range(CT):
                    sq = tr_pool.tile([128, T], BF16, tag="trb0", name=named("t"))
                    nc.scalar.square(sq[:], x_tiles[ci][:])
                    nc.tensor.matmul(
                        ps_ss[0:1, :], ones_col[:], sq[:],
                        start=(ci == 0), stop=(ci == CT - 1),
                    )
            s_sb = sm_pool.tile([1, T], F32, tag="s1", name=named("t"))
            nc.scalar.activation(
                s_sb[:], ps_ss[0:1, :], mybir.ActivationFunctionType.Ln,
                bias=eps_t[:], scale=1.0 / C,
            )
            rstd = sm_pool.tile([1, T], BF16, tag="s2", name=named("t"))
            nc.scalar.activation(
                rstd[:], s_sb[:], mybir.ActivationFunctionType.Exp,
                scale=-0.5,
            )
            # broadcast rstd across all 128 partitions once (the baseline
            # did one nw*rstd broadcast matmul per tile; the norm weight is
            # folded into the weights on the host now)
            ps_bc = ps_tile()
            nc.tensor.matmul(ps_bc[:, :], ones_row[0:1, :], rstd[:],
                             start=True, stop=True)
            out = []
            for ci in range(CT):
                h = ht_pool.tile([128, T], BF16, tag=out_tag, name=named("t"))
                nc.vector.tensor_mul(h[:], x_tiles[ci][:], ps_bc[:, :])
                out.append(h)
            return out

        # ---- rmsnorm 1
        ht = rmsnorm(xt, 0, "ht")

        # ---- q/k projection (T layout) + RoPE
        qkrot = [None] * (QKN // 128)
        n_och = QKN // 128
        OGQ = 4
        rope_pend = []

        def flush_rope(n=99):
            # emitted one og-group late (dripped between matmul bursts) so
            # the swap matmul's inputs are ready and the DVE work is spread
            for _ in range(min(n, len(rope_pend))):
                idx, src = rope_pend.pop(0)
                a = tr_pool.tile([128, T], BF16, tag="trb2", name=named("t"))
                nc.vector.tensor_mul(a[:], src[:], sb_cos[:])
                m = tr_pool.tile([128, T], BF16, tag="trb3", name=named("t"))
                nc.vector.tensor_mul(m[:], src[:], sb_nsin[:])
                ps_b = ps_tile()
                nc.tensor.matmul(ps_b[:, :], sb_psw[:], m[:],
                                 start=True, stop=True)
                rot = qk_pool.tile([128, T], BF16, tag="qk", name=named("t"))
                nc.vector.tensor_add(rot[:], a[:], ps_b[:, :])
                qkrot[idx] = rot

        for og in range(0, n_och, OGQ):
            g = min(OGQ, n_och - og)
            pss = [ps_tile() for _ in range(g)]
            for ci in range(CT):
                wt = wsl_pool.tile([128, OG * 128], BF16, tag="wsl", name=named("t"))
                eng = nc.sync if ci % 2 == 0 else nc.scalar
                eng.dma_start(
                    wt[:, :g * 128],
                    w_qkT[ci * 128:(ci + 1) * 128, og * 128:(og + g) * 128],
                )
                for j in range(g):
                    nc.tensor.matmul(
                        pss[j][:, :],
                        wt[:, j * 128:(j + 1) * 128],
                        ht[ci][:],
                        start=(ci == 0), stop=(ci == CT - 1),
                    )
                if ci % 4 == 3:
                    flush_rope(1)
            flush_rope(99)  # safety: CT may be < 16
            nxt = []
            for j in range(g):
                src = rsrc_pool.tile([128, T], BF16, tag="ropesrc", name=named("t"))
                nc.scalar.copy(src[:], pss[j][:, :])
                nxt.append((og + j, src))
            rope_pend = nxt
        flush_rope()

        # ---- v projection interleaved with attention: each dv group
        # produces the v columns for its 8 heads, then those heads run.
        # Dense v-proj matmul bursts keep the PE HAM warm between the
        # sparser attention head chains; paired heads (base partitions
        # 0/64) put their score matmuls on different PE row groups.
        v1 = [v_pool.tile([128, H * 65], BF16, tag="v1", name=named("t")) for _ in range(KT)]
        for tc_i in range(KT):
            ones_slots = v1[tc_i].rearrange("p (h e) -> p h e", e=65)[:, :, 64]
            nc.vector.memset(ones_slots, 1.0)
        DVW = min(512, C)
        hpg = DVW // D  # heads per dv chunk

        ctxT = [ctx_pool.tile([128, T], BF16, tag="ctx", name=named("t")) for _ in range(CT)]
        pend = []

        def flush_head():
            """64-lane softmax normalization: broadcast the denominator row
            (already in SBUF via the ctx copy) to 64 partitions on the PE,
            then Ln/Exp run as [64,T] scalar ops (~0.35us) instead of
            single-lane [1,T] ops (~0.7us) — the scalar queue paces
            attention."""
            h, cs = pend.pop(0)
            ps_d = ps_tile()
            nc.tensor.matmul(
                ps_d[0:D, :], ones_rows[64:65, 0:D], cs[64:65, :],
                start=True, stop=True,
            )
            dl = sm_pool.tile([64, T], F32, tag="dl64", bufs=2, name=named("t"))
            nc.scalar.activation(
                dl[:], ps_d[0:D, :], mybir.ActivationFunctionType.Ln,
            )
            rec = sm_pool.tile([64, T], BF16, tag="rec64", bufs=2, name=named("t"))
            nc.scalar.activation(
                rec[:], dl[:], mybir.ActivationFunctionType.Exp, scale=-1.0,
            )
            co = (h * D) % 128
            nc.vector.tensor_mul(
                ctxT[(h * D) // 128][co:co + D, :],
                cs[0:D, :], rec[:],
            )

        def attn_head_pair(h0):
            hs = [h0, h0 + 1] if h0 + 1 < H else [h0]
            ctxps = {}
            es = {}
            for h in hs:
                ctxps[h] = ps_tile()
                es[h] = []
            for kt in range(KT):
                ncols = T - kt * 128
                for h in hs:
                    q_t = qkrot[(h * D) // 128]
                    k_t = qkrot[(C + h * D) // 128]
                    ro = (h * D) % 128
                    s_ps = ps_tile()
                    nc.tensor.matmul(
                        s_ps[:, :ncols],
                        k_t[ro:ro + D, kt * 128:(kt + 1) * 128],
                        q_t[ro:ro + D, kt * 128:],
                        start=True, stop=True,
                    )
                    e_sb = e_pool.tile([128, T], BF16, tag="e", name=named("t"))
                    nc.scalar.activation(
                        e_sb[:, :ncols], s_ps[:, :ncols],
                        mybir.ActivationFunctionType.Exp, scale=scale,
                    )
                    nc.vector.tensor_mul(
                        e_sb[:, 0:128], e_sb[:, 0:128], sb_tri[:],
                    )
                    es[h].append(e_sb)
            for kt in range(KT):
                ncols = T - kt * 128
                for h in hs:
                    nc.tensor.matmul(
                        ctxps[h][0:65, kt * 128:],
                        v1[kt][:, h * 65:(h + 1) * 65],
                        es[h][kt][:, :ncols],
                        start=(kt == 0), stop=(kt == KT - 1),
                        skip_group_check=True,
                    )
            for h in hs:
                cs = tr_pool.tile([128, T], BF16, tag="trb1", name=named("t"))
                nc.vector.tensor_copy(cs[0:65, :], ctxps[h][0:65, :])
                pend.append((h, cs))
            while len(pend) > 2:
                flush_head()

        for dv in range(C // DVW):
            pss = [ps_tile() for _ in range(KT)]
            for ci in range(CT):
                wt = wsl_pool.tile([128, OG * 128], BF16, tag="wsl", name=named("t"))
                nc.sync.dma_start(
                    wt[:, :DVW],
                    w_vT[ci * 128:(ci + 1) * 128, dv * DVW:(dv + 1) * DVW],
                )
                for tc_i in range(KT):
                    nc.tensor.matmul(
                        pss[tc_i][:, :DVW],
                        ht[ci][:, tc_i * 128:(tc_i + 1) * 128],
                        wt[:, :DVW],
                        start=(ci == 0), stop=(ci == CT - 1),
                    )
            h0 = dv * hpg
            for tc_i in range(KT):
                dst = v1[tc_i][:, h0 * 65:(h0 + hpg) * 65].rearrange(
                    "p (h e) -> p h e", e=65)[:, :, 0:64]
                srcap = pss[tc_i][:, :DVW].rearrange("p (h e) -> p h e", e=64)
                nc.scalar.copy(dst, srcap)
            for hh in range(h0, h0 + hpg, 2):
                attn_head_pair(hh)
        while pend:
            flush_head()

        # ---- o-proj (T layout) + residual -> x2T
        x2t = [None] * CT
        OGO = 4
        ps_ss2 = ps_tile()
        for og in range(0, CT, OGO):
            g = min(OGO, CT - og)
            pss = [ps_tile() for _ in range(g)]
            for ci in range(CT):
                wt = wsl_pool.tile([128, OG * 128], BF16, tag="wsl", name=named("t"))
                eng = nc.sync if ci % 2 == 0 else nc.scalar
                eng.dma_start(
                    wt[:, :g * 128],
                    w_oT[ci * 128:(ci + 1) * 128, og * 128:(og + g) * 128],
                )
                for j in range(g):
                    nc.tensor.matmul(
                        pss[j][:, :],
                        wt[:, j * 128:(j + 1) * 128],
                        ctxT[ci][:],
                        start=(ci == 0), stop=(ci == CT - 1),
                    )
            for j in range(g):
                x2 = xt_pool.tile([128, T], F32, tag="xt", name=named("t"))
                nc.vector.tensor_add(x2[:], xt[og + j][:], pss[j][:, :])
                x2t[og + j] = x2
                sq2 = tr_pool.tile([128, T], BF16, tag="trb0", name=named("t"))
                nc.scalar.square(sq2[:], x2[:])
                nc.tensor.matmul(
                    ps_ss2[0:1, :], ones_col[:], sq2[:],
                    start=(og + j == 0), stop=(og + j == CT - 1),
                )

        # ---- rmsnorm 2
        h2t = rmsnorm(x2t, 1, "ht", ps_ss=ps_ss2)

        # ---- FFN up + swiglu -> actT (bf16, I rows)
        actT = [None] * ICH
        GG = min(4, ICH)  # gate chunks per group (paired with value chunks)
        for gg in range(0, ICH, GG):
            g = min(GG, ICH - gg)
            ps_gate = [ps_tile() for _ in range(g)]
            ps_val = [ps_tile() for _ in range(g)]
            for ci in range(CT):
                wt = wsl_pool.tile([128, OG * 128], BF16, tag="wsl", name=named("t"))
                nc.sync.dma_start(
                    wt[:, :g * 128],
                    w_upT[ci * 128:(ci + 1) * 128, gg * 128:(gg + g) * 128],
                )
                nc.scalar.dma_start(
                    wt[:, GG * 128:(GG + g) * 128],
                    w_upT[ci * 128:(ci + 1) * 128,
                          I + gg * 128:I + (gg + g) * 128],
                )
                for j in range(g):
                    nc.tensor.matmul(
                        ps_gate[j][:, :], wt[:, j * 128:(j + 1) * 128],
                        h2t[ci][:],
                        start=(ci == 0), stop=(ci == CT - 1),
                    )
                    nc.tensor.matmul(
                        ps_val[j][:, :],
                        wt[:, (GG + j) * 128:(GG + j + 1) * 128],
                        h2t[ci][:],
                        start=(ci == 0), stop=(ci == CT - 1),
                    )
            for j in range(g):
                sg = tr_pool.tile([128, T], BF16, tag="trb1", name=named("t"))
                nc.scalar.activation(
                    sg[:], ps_gate[j][:, :],
                    mybir.ActivationFunctionType.Silu,
                )
                a = qk_pool.tile([128, T], BF16, tag="qk", name=named("t"))
                nc.vector.tensor_mul(a[:], sg[:], ps_val[j][:, :])
                actT[gg + j] = a

        # ---- FFN down + residual -> outT
        OGD = 4
        for og in range(0, CT, OGD):
            g = min(OGD, CT - og)
            pss = [ps_tile() for _ in range(g)]
            for ii in range(ICH):
                wt = wsl_pool.tile([128, OG * 128], BF16, tag="wsl", name=named("t"))
                eng = nc.sync if ii % 2 == 0 else nc.scalar
                eng.dma_start(
                    wt[:, :g * 128],
                    w_downT[ii * 128:(ii + 1) * 128, og * 128:(og + g) * 128],
                )
                for j in range(g):
                    nc.tensor.matmul(
                        pss[j][:, :],
                        wt[:, j * 128:(j + 1) * 128],
                        actT[ii][:],
                        start=(ii == 0), stop=(ii == ICH - 1),
                    )
            for j in range(g):
                o_sb = tr_pool.tile([128, T], F32, tag="trf", name=named("t"))
                nc.vector.tensor_add(o_sb[:], x2t[og + j][:], pss[j][:, :])
                eng2 = nc.sync if j % 2 == 0 else nc.scalar
                eng2.dma_start(
                    outT[(og + j) * 128:(og + j + 1) * 128, :], o_sb[:],
                )

    _split_excess_waits(nc)
    return nc


def make_core_inputs(cfg: Cfg, x_shard, w_qkv, w_o, w_up, w_down,
                     attn_norm_w, ffn_norm_w, pos0, shared):
    """Host-side prep of one core's input map. x_shard [T, C] fp32.
    `shared` caches the (identical) weight arrays across cores."""
    T, C, D = cfg.T, cfg.C, cfg.D
    if not shared:
        nw1 = attn_norm_w.astype(np.float32)[:, None]   # [C, 1]
        nw2 = ffn_norm_w.astype(np.float32)[:, None]
        shared["w_qkT"] = np.ascontiguousarray(
            w_qkv[:2 * C].T * nw1).astype(NPBF16)
        shared["w_vT"] = np.ascontiguousarray(
            w_qkv[2 * C:3 * C].T * nw1).astype(NPBF16)
        shared["w_oT"] = np.ascontiguousarray(w_o.T).astype(NPBF16)
        shared["w_upT"] = np.ascontiguousarray(w_up.T * nw2).astype(NPBF16)
        shared["w_downT"] = np.ascontiguousarray(w_down.T).astype(NPBF16)
        k_idx = np.arange(128)
        shared["trimask"] = (
            k_idx[:, None] <= k_idx[None, :]).astype(NPBF16)
        psw = np.zeros((128, 128), dtype=NPBF16)
        psw[k_idx ^ 32, k_idx] = 1.0  # lhsT[j, p] = 1 iff j == p ^ 32
        shared["pswap"] = psw
    inv = (1.0 / ROPE_THETA ** (np.arange(0, D, 2) / D)).astype(np.float64)
    pos = np.arange(pos0, pos0 + T, dtype=np.float64)
    fr = np.outer(pos, inv)                       # [T, D/2]
    emb = np.concatenate([fr, fr], axis=-1)       # [T, D]
    cosT = np.cos(emb).T.astype(np.float32)       # [D, T]
    sinT = np.sin(emb).T.astype(np.float32)
    nsinT = sinT.copy()
    nsinT[:D // 2] *= -1.0
    reps = 128 // D
    nsin2 = np.tile(nsinT, (reps, 1))
    perm = np.arange(128) ^ 32
    s2 = nsin2[perm]          # s2[p] = nsin2[p ^ 32]
    return {
        "xT": np.ascontiguousarray(x_shard.T).astype(np.float32),
        "cosT2": np.tile(cosT, (reps, 1)).astype(NPBF16),
        "nsinT2": s2.astype(NPBF16),
        **shared,
    }


def kernel(x, attn_norm_w, ffn_norm_w, w_qkv, w_o, w_up, w_down,
           _trace=False, _tmpdir=None):
    x = np.asarray(x, dtype=np.float32)
    attn_norm_w = np.asarray(attn_norm_w, dtype=np.float32)
    ffn_norm_w = np.asarray(ffn_norm_w, dtype=np.float32)
    w_qkv = np.asarray(w_qkv, dtype=np.float32)
    w_o = np.asarray(w_o, dtype=np.float32)
    w_up = np.asarray(w_up, dtype=np.float32)
    w_down = np.asarray(w_down, dtype=np.float32)

    B, S, C = x.shape
    cfg = Cfg(T=512, C=C, H=C // 64, D=64, I=2 * C)
    n_blocks = S // cfg.T
    assert B * n_blocks == 8

    nc = build_program(cfg)

    shared = {}
    in_maps = []
    for core in range(8):
        b, blk = divmod(core, n_blocks)
        sl = slice(blk * cfg.T, (blk + 1) * cfg.T)
        in_maps.append(make_core_inputs(
            cfg, x[b, sl], w_qkv, w_o, w_up, w_down,
            attn_norm_w, ffn_norm_w, pos0=blk * cfg.T, shared=shared,
        ))

    res = run_bass_kernel_spmd(
        nc, in_maps, core_ids=list(range(8)),
        trace=_trace, tmpdir=_tmpdir,
    )

    out = np.empty((B, S, C), dtype=np.float32)
    for core in range(8):
        b, blk = divmod(core, n_blocks)
        sl = slice(blk * cfg.T, (blk + 1) * cfg.T)
        out[b, sl] = res.results[core]["outT"].T
    kernel.last_result = res
    return out



# revision 43
# speedup vs baseline: 1.0263x; 1.0263x over previous
"""Trainium2 Bass kernel for nn_BlockAttentionResidual.

Transformer block: RMSNorm -> QKV -> RoPE -> block-diagonal causal attention
(4 blocks of 512) -> o-proj + residual -> RMSNorm -> SwiGLU FFN + residual.
Shapes: x [2, 2048, 2048], 32 heads x 64, inter 4096.

Sharding: 8 cores = (batch 2) x (4 sequence blocks of 512 tokens). The
attention mask is block-diagonal causal with block size 512, so each core's
512-token slice is fully independent -> no collectives.

On-device layout is "T layout" [feature, token] throughout, because every
matmul contracts the feature dim, which must sit on SBUF partitions for the
PE. Matmuls run in bf16 with fp32 PSUM accumulation; softmax skips the max
subtraction (scores are small); the causal mask is only needed on the
128x128 diagonal chunks of each 512 block; softmax denominators come from a
ones-column appended to V; per-token broadcasts across partitions are done
as rank-1 outer-product matmuls on the PE.
"""

import math
from contextlib import ExitStack

import ml_dtypes
import numpy as np

import concourse.bass as bass
import concourse.mybir as mybir
import concourse.tile as tile
from concourse.bass_utils import run_bass_kernel_spmd
from concourse.vector_clock import ScopedClock

F32 = mybir.dt.float32
BF16 = mybir.dt.bfloat16
NPBF16 = ml_dtypes.bfloat16

EPS = 1e-5
ROPE_THETA = 10000.0


# --- workaround: this walrus build allows only one sem wait per CTRL-queue
# instruction (Drain/NoOp), but Tile's tail drain aggregates every
# outstanding wait onto a single SP Drain. Spread them over SP NOPs.
def _patched_drain_and_barrier(self, tick_clock, wait_clock):
    nop_inst = self.nc.sync.nop(nofuse=True)
    wait_clock.add_sem_waits(
        nop_inst.ins, ScopedClock({None: tick_clock.global_clock})
    )
    si = nop_inst.ins.sync_info
    waits = list(si.on_wait) if si is not None else []
    if len(waits) > 1:
        si.on_wait = waits[:1]
        for w in waits[1:]:
            n2 = self.nc.sync.nop(nofuse=True)
            if n2.ins.sync_info is None:
                n2.ins.sync_info = mybir.SyncInfo(on_wait=[w], on_update=[])
            else:
                n2.ins.sync_info.on_wait = [w]
    self.nc.sync.drain()
    self.nc.all_engine_barrier()
    assert self.sems is not None
    popped = self.nc._tile_sem_poison_stack.pop()
    assert popped is self._sem_poison
    self.nc.clear_and_free_semaphores(list(self.sems.allocated().values()))
    self.nc.all_engine_barrier()


tile.TileContext._drain_and_barrier = _patched_drain_and_barrier


def _split_excess_waits(nc, maxw=1):
    """This walrus build rejects instructions carrying more than one sync
    wait; hoist extras onto single-wait NOPs queued just before on the same
    engine."""
    fn = nc.m.functions[0]
    for bb in fn.blocks:
        out = []
        changed = False
        for inst in bb.instructions:
            si = getattr(inst, "sync_info", None)
            waits = list(si.on_wait) if si is not None else []
            if len(waits) > maxw:
                changed = True
                for w in waits[:-maxw]:
                    nop = mybir.InstNoOp(
                        name=nc.get_next_instruction_name(), ins=[], outs=[])
                    nop.engine = inst.engine
                    nop.sync_info = mybir.SyncInfo(on_wait=[w], on_update=[])
                    out.append(nop)
                si.on_wait = waits[-maxw:]
            out.append(inst)
        if changed:
            bb.instructions = out


class Cfg:
    def __init__(self, T=512, C=2048, H=32, D=64, I=4096):
        self.T = T          # tokens per core (one attention block)
        self.C = C          # hidden
        self.H = H          # heads
        self.D = D          # head dim (must be 64)
        self.I = I          # ffn inner
        assert D == 64 and C == H * D
        assert T % 128 == 0 and C % 128 == 0 and I % 128 == 0


def build_program(cfg: Cfg):
    T, C, H, D, I = cfg.T, cfg.C, cfg.H, cfg.D, cfg.I
    CT = C // 128            # hidden tiles
    KT = T // 128            # token chunks (and attention k-chunks)
    QKN = 2 * C              # q+k feature rows
    ICH = I // 128           # ffn inner chunks
    scale = 1.0 / math.sqrt(D)
    OG = 8                   # psum group width for dense matmul phases

    nc = bass.Bass("TRN2", target_bir_lowering=False, debug=False)

    xT = nc.dram_tensor("xT", (C, T), F32, kind="ExternalInput").ap()
    w_qkT = nc.dram_tensor("w_qkT", (C, QKN), BF16, kind="ExternalInput").ap()
    w_vT = nc.dram_tensor("w_vT", (C, C), BF16, kind="ExternalInput").ap()
    w_oT = nc.dram_tensor("w_oT", (C, C), BF16, kind="ExternalInput").ap()
    w_upT = nc.dram_tensor("w_upT", (C, 2 * I), BF16, kind="ExternalInput").ap()
    w_downT = nc.dram_tensor("w_downT", (I, C), BF16, kind="ExternalInput").ap()
    cosT2 = nc.dram_tensor("cosT2", (128, T), BF16, kind="ExternalInput").ap()
    nsinT2 = nc.dram_tensor("nsinT2", (128, T), BF16, kind="ExternalInput").ap()
    trimask = nc.dram_tensor("trimask", (128, 128), BF16, kind="ExternalInput").ap()
    pswap = nc.dram_tensor("pswap", (128, 128), BF16, kind="ExternalInput").ap()
    outT = nc.dram_tensor("outT", (C, T), F32, kind="ExternalOutput").ap()

    with tile.TileContext(nc) as tc, ExitStack() as ctx:
        consts = ctx.enter_context(tc.tile_pool(name="consts", bufs=1))
        # xt slots also serve x2 (x dies at the o-proj residual add); +2
        # rolling slots so the x2 alloc never waits on its own free.
        xt_pool = ctx.enter_context(tc.tile_pool(name="xt", bufs=CT + 2))
        ht_pool = ctx.enter_context(tc.tile_pool(name="ht", bufs=CT))
        qk_pool = ctx.enter_context(
            tc.tile_pool(name="qk", bufs=max(2 * CT, ICH)))
        v_pool = ctx.enter_context(tc.tile_pool(name="v", bufs=KT))
        ctx_pool = ctx.enter_context(tc.tile_pool(name="ctx", bufs=CT))
        wsl_pool = ctx.enter_context(tc.tile_pool(name="wsl", bufs=8))
        tr_pool = ctx.enter_context(tc.tile_pool(name="tr", bufs=4))
        e_pool = ctx.enter_context(tc.tile_pool(name="e", bufs=8))
        sm_pool = ctx.enter_context(tc.tile_pool(name="sm", bufs=2))
        rsrc_pool = ctx.enter_context(tc.tile_pool(name="rsrc", bufs=9))
        ps_pool = ctx.enter_context(
            tc.tile_pool(name="ps", bufs=8, space="PSUM"))

        _nm = [0]

        def named(base):
            _nm[0] += 1
            return f"{base}{_nm[0]}"

        def ps_tile():
            return ps_pool.tile([128, T], F32, tag="ps", name=named("ps"))

        # ---- load xT first: the first sumsq matmul waits on xt[0], so
        # its DMA must lead the issue queues (constants aren't needed until
        # RoPE, ~40us in)
        xt = []
        for ci in range(CT):
            t = xt_pool.tile([128, T], F32, tag="xt", name=named("t"))
            nc.sync.dma_start(t[:], xT[ci * 128:(ci + 1) * 128, :])
            xt.append(t)

        # ---- constants
        sb_cos = consts.tile([128, T], BF16)
        nc.sync.dma_start(sb_cos[:], cosT2[:])
        sb_nsin = consts.tile([128, T], BF16)
        nc.sync.dma_start(sb_nsin[:], nsinT2[:])
        sb_tri = consts.tile([128, 128], BF16)
        nc.sync.dma_start(sb_tri[:], trimask[:])
        sb_psw = consts.tile([128, 128], BF16)
        nc.sync.dma_start(sb_psw[:], pswap[:])
        ones_col = consts.tile([128, 1], BF16)
        nc.vector.memset(ones_col[:], 1.0)
        ones_row = consts.tile([1, 128], BF16)
        nc.vector.memset(ones_row[:], 1.0)
        eps_t = consts.tile([1, 1], F32)
        nc.vector.memset(eps_t[:], EPS)


        def rmsnorm(x_tiles, nw_row, out_tag, ps_ss=None):
            """x_tiles: CT fp32 [128, T] tiles (T layout) -> bf16 tiles of
            x * rstd[t] (the norm weight is folded into the next projection
            on the host). ps_ss: optionally a psum tile already holding the
            sum of squares."""
            if ps_ss is None:
                ps_ss = ps_tile()
                for ci in range(CT):
                    sq = tr_pool.tile([128, T], BF16, tag="trb0", name=named("t"))
                    nc.scalar.square(sq[:], x_tiles[ci][:])
                    nc.tensor.matmul(
                        ps_ss[0:1, :], ones_col[:], sq[:],
                        start=(ci == 0), stop=(ci == CT - 1),
                    )
            s_sb = sm_pool.tile([1, T], F32, tag="s1", name=named("t"))
            nc.scalar.activation(
                s_sb[:], ps_ss[0:1, :], mybir.ActivationFunctionType.Ln,
                bias=eps_t[:], scale=1.0 / C,
            )
            rstd = sm_pool.tile([1, T], BF16, tag="s2", name=named("t"))
            nc.scalar.activation(
                rstd[:], s_sb[:], mybir.ActivationFunctionType.Exp,
                scale=-0.5,
            )
            # broadcast rstd across all 128 partitions once (the baseline
            # did one nw*rstd broadcast matmul per tile; the norm weight is
            # folded into the weights on the host now)
            ps_bc = ps_tile()
            nc.tensor.matmul(ps_bc[:, :], ones_row[0:1, :], rstd[:],
                             start=True, stop=True)
            out = []
            for ci in range(CT):
                h = ht_pool.tile([128, T], BF16, tag=out_tag, name=named("t"))
                nc.vector.tensor_mul(h[:], x_tiles[ci][:], ps_bc[:, :])
                out.append(h)
            return out

        # ---- rmsnorm 1
        ht = rmsnorm(xt, 0, "ht")

        # ---- q/k projection (T layout) + RoPE
        qkrot = [None] * (QKN // 128)
        n_och = QKN // 128
        OGQ = 4
        rope_pend = []

        def flush_rope(n=99):
            # emitted one og-group late (dripped between matmul bursts) so
            # the swap matmul's inputs are ready and the DVE work is spread
            for _ in range(min(n, len(rope_pend))):
                idx, src = rope_pend.pop(0)
                a = tr_pool.tile([128, T], BF16, tag="trb2", name=named("t"))
                nc.vector.tensor_mul(a[:], src[:], sb_cos[:])
                m = tr_pool.tile([128, T], BF16, tag="trb3", name=named("t"))
                nc.vector.tensor_mul(m[:], src[:], sb_nsin[:])
                ps_b = ps_tile()
                nc.tensor.matmul(ps_b[:, :], sb_psw[:], m[:],
                                 start=True, stop=True)
                rot = qk_pool.tile([128, T], BF16, tag="qk", name=named("t"))
                nc.vector.tensor_add(rot[:], a[:], ps_b[:, :])
                qkrot[idx] = rot

        for og in range(0, n_och, OGQ):
            g = min(OGQ, n_och - og)
            pss = [ps_tile() for _ in range(g)]
            for ci in range(CT):
                wt = wsl_pool.tile([128, OG * 128], BF16, tag="wsl", name=named("t"))
                eng = nc.sync if ci % 2 == 0 else nc.scalar
                eng.dma_start(
                    wt[:, :g * 128],
                    w_qkT[ci * 128:(ci + 1) * 128, og * 128:(og + g) * 128],
                )
                for j in range(g):
                    nc.tensor.matmul(
                        pss[j][:, :],
                        wt[:, j * 128:(j + 1) * 128],
                        ht[ci][:],
                        start=(ci == 0), stop=(ci == CT - 1),
                    )
                if ci % 4 == 3:
                    flush_rope(1)
            flush_rope(99)  # safety: CT may be < 16
            nxt = []
            for j in range(g):
                src = rsrc_pool.tile([128, T], BF16, tag="ropesrc", name=named("t"))
                nc.scalar.copy(src[:], pss[j][:, :])
                nxt.append((og + j, src))
            rope_pend = nxt
        flush_rope()

        # ---- v projection interleaved with attention: each dv group
        # produces the v columns for its 8 heads, then those heads run.
        # Dense v-proj matmul bursts keep the PE HAM warm between the
        # sparser attention head chains; paired heads (base partitions
        # 0/64) put their score matmuls on different PE row groups.
        v1 = [v_pool.tile([128, H * 65], BF16, tag="v1", name=named("t")) for _ in range(KT)]
        for tc_i in range(KT):
            ones_slots = v1[tc_i].rearrange("p (h e) -> p h e", e=65)[:, :, 64]
            nc.vector.memset(ones_slots, 1.0)
        DVW = min(512, C)
        hpg = DVW // D  # heads per dv chunk

        ctxT = [ctx_pool.tile([128, T], BF16, tag="ctx", name=named("t")) for _ in range(CT)]
        pend = []

        def flush_head():
            h, ctx_ps = pend.pop(0)
            dl = sm_pool.tile([1, T], F32, tag="dl", name=named("t"))
            nc.scalar.activation(
                dl[:], ctx_ps[64:65, :], mybir.ActivationFunctionType.Ln,
            )
            rec = sm_pool.tile([1, T], BF16, tag="rec", name=named("t"))
            nc.scalar.activation(
                rec[:], dl[:], mybir.ActivationFunctionType.Exp, scale=-1.0,
            )
            ps_rec = ps_tile()
            nc.tensor.matmul(
                ps_rec[0:D, :], ones_row[0:1, 0:D], rec[:],
                start=True, stop=True,
            )
            cs = tr_pool.tile([128, T], BF16, tag="trb1", name=named("t"))
            nc.vector.tensor_copy(cs[0:D, :], ctx_ps[0:D, :])
            co = (h * D) % 128
            nc.vector.tensor_mul(
                ctxT[(h * D) // 128][co:co + D, :],
                cs[0:D, :], ps_rec[0:D, :],
            )

        def attn_head_pair(h0):
            hs = [h0, h0 + 1] if h0 + 1 < H else [h0]
            ctxps = {}
            es = {}
            for h in hs:
                ctxps[h] = ps_tile()
                es[h] = []
            for kt in range(KT):
                ncols = T - kt * 128
                for h in hs:
                    q_t = qkrot[(h * D) // 128]
                    k_t = qkrot[(C + h * D) // 128]
                    ro = (h * D) % 128
                    s_ps = ps_tile()
                    nc.tensor.matmul(
                        s_ps[:, :ncols],
                        k_t[ro:ro + D, kt * 128:(kt + 1) * 128],
                        q_t[ro:ro + D, kt * 128:],
                        start=True, stop=True,
                    )
                    e_sb = e_pool.tile([128, T], BF16, tag="e", name=named("t"))
                    nc.scalar.activation(
                        e_sb[:, :ncols], s_ps[:, :ncols],
                        mybir.ActivationFunctionType.Exp, scale=scale,
                    )
                    nc.vector.tensor_mul(
                        e_sb[:, 0:128], e_sb[:, 0:128], sb_tri[:],
                    )
                    es[h].append(e_sb)
            for kt in range(KT):
                ncols = T - kt * 128
                for h in hs:
                    nc.tensor.matmul(
                        ctxps[h][0:65, kt * 128:],
                        v1[kt][:, h * 65:(h + 1) * 65],
                        es[h][kt][:, :ncols],
                        start=(kt == 0), stop=(kt == KT - 1),
                        skip_group_check=True,
                    )
            for h in hs:
                pend.append((h, ctxps[h]))
            while len(pend) > 2:
                flush_head()

        for dv in range(C // DVW):
            pss = [ps_tile() for _ in range(KT)]
            for ci in range(CT):
                wt = wsl_pool.tile([128, OG * 128], BF16, tag="wsl", name=named("t"))
                nc.sync.dma_start(
                    wt[:, :DVW],
                    w_vT[ci * 128:(ci + 1) * 128, dv * DVW:(dv + 1) * DVW],
                )
                for tc_i in range(KT):
                    nc.tensor.matmul(
                        pss[tc_i][:, :DVW],
                        ht[ci][:, tc_i * 128:(tc_i + 1) * 128],
                        wt[:, :DVW],
                        start=(ci == 0), stop=(ci == CT - 1),
                    )
            h0 = dv * hpg
            for tc_i in range(KT):
                dst = v1[tc_i][:, h0 * 65:(h0 + hpg) * 65].rearrange(
                    "p (h e) -> p h e", e=65)[:, :, 0:64]
                srcap = pss[tc_i][:, :DVW].rearrange("p (h e) -> p h e", e=64)
                nc.scalar.copy(dst, srcap)
            for hh in range(h0, h0 + hpg, 2):
                attn_head_pair(hh)
        while pend:
            flush_head()

        # ---- o-proj (T layout) + residual -> x2T
        x2t = [None] * CT
        OGO = 4
        ps_ss2 = ps_tile()
        for og in range(0, CT, OGO):
            g = min(OGO, CT - og)
            pss = [ps_tile() for _ in range(g)]
            for ci in range(CT):
                wt = wsl_pool.tile([128, OG * 128], BF16, tag="wsl", name=named("t"))
                eng = nc.sync if ci % 2 == 0 else nc.scalar
                eng.dma_start(
                    wt[:, :g * 128],
                    w_oT[ci * 128:(ci + 1) * 128, og * 128:(og + g) * 128],
                )
                for j in range(g):
                    nc.tensor.matmul(
                        pss[j][:, :],
                        wt[:, j * 128:(j + 1) * 128],
                        ctxT[ci][:],
                        start=(ci == 0), stop=(ci == CT - 1),
                    )
            for j in range(g):
                x2 = xt_pool.tile([128, T], F32, tag="xt", name=named("t"))
                nc.vector.tensor_add(x2[:], xt[og + j][:], pss[j][:, :])
                x2t[og + j] = x2
                sq2 = tr_pool.tile([128, T], BF16, tag="trb0", name=named("t"))
                nc.scalar.square(sq2[:], x2[:])
                nc.tensor.matmul(
                    ps_ss2[0:1, :], ones_col[:], sq2[:],
                    start=(og + j == 0), stop=(og + j == CT - 1),
                )

        # ---- rmsnorm 2
        h2t = rmsnorm(x2t, 1, "ht", ps_ss=ps_ss2)

        # ---- FFN up + swiglu -> actT (bf16, I rows)
        actT = [None] * ICH
        GG = min(4, ICH)  # gate chunks per group (paired with value chunks)
        for gg in range(0, ICH, GG):
            g = min(GG, ICH - gg)
            ps_gate = [ps_tile() for _ in range(g)]
            ps_val = [ps_tile() for _ in range(g)]
            for ci in range(CT):
                wt = wsl_pool.tile([128, OG * 128], BF16, tag="wsl", name=named("t"))
                nc.sync.dma_start(
                    wt[:, :g * 128],
                    w_upT[ci * 128:(ci + 1) * 128, gg * 128:(gg + g) * 128],
                )
                nc.scalar.dma_start(
                    wt[:, GG * 128:(GG + g) * 128],
                    w_upT[ci * 128:(ci + 1) * 128,
                          I + gg * 128:I + (gg + g) * 128],
                )
                for j in range(g):
                    nc.tensor.matmul(
                        ps_gate[j][:, :], wt[:, j * 128:(j + 1) * 128],
                        h2t[ci][:],
                        start=(ci == 0), stop=(ci == CT - 1),
                    )
                    nc.tensor.matmul(
                        ps_val[j][:, :],
                        wt[:, (GG + j) * 128:(GG + j + 1) * 128],
                        h2t[ci][:],
                        start=(ci == 0), stop=(ci == CT - 1),
                    )
            for j in range(g):
                sg = tr_pool.tile([128, T], BF16, tag="trb1", name=named("t"))
                nc.scalar.activation(
                    sg[:], ps_gate[j][:, :],
                    mybir.ActivationFunctionType.Silu,
                )
                a = qk_pool.tile([128, T], BF16, tag="qk", name=named("t"))
                nc.vector.tensor_mul(a[:], sg[:], ps_val[j][:, :])
                actT[gg + j] = a

        # ---- FFN down + residual -> outT
        OGD = 4
        for og in range(0, CT, OGD):
            g = min(OGD, CT - og)
            pss = [ps_tile() for _ in range(g)]
            for ii in range(ICH):
                wt = wsl_pool.tile([128, OG * 128], BF16, tag="wsl", name=named("t"))
                eng = nc.sync if ii % 2 == 0 else nc.scalar
                eng.dma_start(
                    wt[:, :g * 128],
                    w_downT[ii * 128:(ii + 1) * 128, og * 128:(og + g) * 128],
                )
                for j in range(g):
                    nc.tensor.matmul(
                        pss[j][:, :],
                        wt[:, j * 128:(j + 1) * 128],
                        actT[ii][:],
                        start=(ii == 0), stop=(ii == ICH - 1),
                    )
            for j in range(g):
                o_sb = tr_pool.tile([128, T], F32, tag="trf", name=named("t"))
                nc.vector.tensor_add(o_sb[:], x2t[og + j][:], pss[j][:, :])
                eng2 = nc.sync if j % 2 == 0 else nc.scalar
                eng2.dma_start(
                    outT[(og + j) * 128:(og + j + 1) * 128, :], o_sb[:],
                )

    _split_excess_waits(nc)
    return nc


def make_core_inputs(cfg: Cfg, x_shard, w_qkv, w_o, w_up, w_down,
                     attn_norm_w, ffn_norm_w, pos0, shared):
    """Host-side prep of one core's input map. x_shard [T, C] fp32.
    `shared` caches the (identical) weight arrays across cores."""
    T, C, D = cfg.T, cfg.C, cfg.D
    if not shared:
        nw1 = attn_norm_w.astype(np.float32)[:, None]   # [C, 1]
        nw2 = ffn_norm_w.astype(np.float32)[:, None]
        shared["w_qkT"] = np.ascontiguousarray(
            w_qkv[:2 * C].T * nw1).astype(NPBF16)
        shared["w_vT"] = np.ascontiguousarray(
            w_qkv[2 * C:3 * C].T * nw1).astype(NPBF16)
        shared["w_oT"] = np.ascontiguousarray(w_o.T).astype(NPBF16)
        shared["w_upT"] = np.ascontiguousarray(w_up.T * nw2).astype(NPBF16)
        shared["w_downT"] = np.ascontiguousarray(w_down.T).astype(NPBF16)
        k_idx = np.arange(128)
        shared["trimask"] = (
            k_idx[:, None] <= k_idx[None, :]).astype(NPBF16)
        psw = np.zeros((128, 128), dtype=NPBF16)
        psw[k_idx ^ 32, k_idx] = 1.0  # lhsT[j, p] = 1 iff j == p ^ 32
        shared["pswap"] = psw
    inv = (1.0 / ROPE_THETA ** (np.arange(0, D, 2) / D)).astype(np.float64)
    pos = np.arange(pos0, pos0 + T, dtype=np.float64)
    fr = np.outer(pos, inv)                       # [T, D/2]
    emb = np.concatenate([fr, fr], axis=-1)       # [T, D]
    cosT = np.cos(emb).T.astype(np.float32)       # [D, T]
    sinT = np.sin(emb).T.astype(np.float32)
    nsinT = sinT.copy()
    nsinT[:D // 2] *= -1.0
    reps = 128 // D
    nsin2 = np.tile(nsinT, (reps, 1))
    perm = np.arange(128) ^ 32
    s2 = nsin2[perm]          # s2[p] = nsin2[p ^ 32]
    return {
        "xT": np.ascontiguousarray(x_shard.T).astype(np.float32),
        "cosT2": np.tile(cosT, (reps, 1)).astype(NPBF16),
        "nsinT2": s2.astype(NPBF16),
        **shared,
    }


def kernel(x, attn_norm_w, ffn_norm_w, w_qkv, w_o, w_up, w_down,
           _trace=False, _tmpdir=None):
    x = np.asarray(x, dtype=np.float32)
    attn_norm_w = np.asarray(attn_norm_w, dtype=np.float32)
    ffn_norm_w = np.asarray(ffn_norm_w, dtype=np.float32)
    w_qkv = np.asarray(w_qkv, dtype=np.float32)
    w_o = np.asarray(w_o, dtype=np.float32)
    w_up = np.asarray(w_up, dtype=np.float32)
    w_down = np.asarray(w_down, dtype=np.float32)

    B, S, C = x.shape
    cfg = Cfg(T=512, C=C, H=C // 64, D=64, I=2 * C)
    n_blocks = S // cfg.T
    assert B * n_blocks == 8

    nc = build_program(cfg)

    shared = {}
    in_maps = []
    for core in range(8):
        b, blk = divmod(core, n_blocks)
        sl = slice(blk * cfg.T, (blk + 1) * cfg.T)
        in_maps.append(make_core_inputs(
            cfg, x[b, sl], w_qkv, w_o, w_up, w_down,
            attn_norm_w, ffn_norm_w, pos0=blk * cfg.T, shared=shared,
        ))

    res = run_bass_kernel_spmd(
        nc, in_maps, core_ids=list(range(8)),
        trace=_trace, tmpdir=_tmpdir,
    )

    out = np.empty((B, S, C), dtype=np.float32)
    for core in range(8):
        b, blk = divmod(core, n_blocks)
        sl = slice(blk * cfg.T, (blk + 1) * cfg.T)
        out[b, sl] = res.results[core]["outT"].T
    kernel.last_result = res
    return out

